# revision 1
# baseline (speedup 1.0000x reference)
"""GraphSAGE 2-layer (mean aggregation) on 8 TRN2 NeuronCores via Bass/Tile.

Sharding: nodes partitioned into 8 contiguous shards (6250 each); each core
owns the edges whose destination lands in its shard.  Host pre-sorts edges by
destination into 128-node windows; aggregation is done on the TensorEngine as
S^T-weighted matmuls over gathered source rows (indirect DMA), with the
1/count mean weights folded into S.  Layer 2 transforms before aggregating
(z = h @ W_l2, 256->128) so both gathers are 128-wide.  One AllGather of z
between the layers; weights replicated.
"""

import numpy as np

N = 50000
E = 800000
D = 128
H = 256
M = 8
NS = N // M          # 6250 nodes per shard
WIN = (NS + 127) // 128   # 49 windows of 128 node slots
NSP = WIN * 128      # 6272 padded shard size
SQRT_HALF = 0.7071067811865476

_CACHE = {}


def _build(T_w):
    import concourse.bacc as bacc
    import concourse.tile as tile
    from concourse import bass, mybir
    from contextlib import ExitStack

    f32 = mybir.dt.float32
    i32 = mybir.dt.int32
    AF = mybir.ActivationFunctionType
    OP = mybir.AluOpType
    T = WIN * T_w

    nc = bacc.Bacc("TRN2", target_bir_lowering=False, debug=False)

    x_ext = nc.dram_tensor("xfull", [N, D], f32, kind="ExternalInput")
    xT_ext = nc.dram_tensor("xT", [128, NSP], f32, kind="ExternalInput")
    esrc_ext = nc.dram_tensor("esrc", [128, T], i32, kind="ExternalInput")
    esrc2_ext = nc.dram_tensor("esrc2", [128, T], i32, kind="ExternalInput")
    erel_ext = nc.dram_tensor("erel", [128, T], f32, kind="ExternalInput")
    ew_ext = nc.dram_tensor("ew", [128, T], f32, kind="ExternalInput")
    wl1_ext = nc.dram_tensor("wl1", [128, 256], f32, kind="ExternalInput")
    wr1_ext = nc.dram_tensor("wr1", [128, 256], f32, kind="ExternalInput")
    wl2_ext = nc.dram_tensor("wl2", [256, 128], f32, kind="ExternalInput")
    wr2_ext = nc.dram_tensor("wr2", [256, 128], f32, kind="ExternalInput")
    b1_ext = nc.dram_tensor("b1c", [128, 2], f32, kind="ExternalInput")
    b2_ext = nc.dram_tensor("b2b", [128, 128], f32, kind="ExternalInput")
    jc_ext = nc.dram_tensor("jc", [128, 128], f32, kind="ExternalInput")
    out_ext = nc.dram_tensor("out", [NS, D], f32, kind="ExternalOutput")

    with tile.TileContext(nc) as tc, ExitStack() as ctx:
        const = ctx.enter_context(tc.tile_pool(name="const", bufs=1))
        meta = ctx.enter_context(tc.tile_pool(name="meta", bufs=1))
        hpool = ctx.enter_context(tc.tile_pool(name="hpool", bufs=1))
        gbuf = ctx.enter_context(tc.tile_pool(name="gbuf", bufs=8))
        spool = ctx.enter_context(tc.tile_pool(name="spool", bufs=6))
        work = ctx.enter_context(tc.tile_pool(name="work", bufs=2))
        pag = ctx.enter_context(tc.tile_pool(name="pag", bufs=2, space="PSUM"))
        ph = ctx.enter_context(tc.tile_pool(name="ph", bufs=2, space="PSUM"))
        pz = ctx.enter_context(tc.tile_pool(name="pz", bufs=2, space="PSUM"))
        po = ctx.enter_context(tc.tile_pool(name="po", bufs=2, space="PSUM"))
        dram = ctx.enter_context(tc.tile_pool(name="dram", bufs=1, space="DRAM"))

        def load(pool, shape, dt, src, nm):
            t = pool.tile(shape, dt, name=nm)
            nc.sync.dma_start(t[:], src)
            return t

        wl1_t = load(const, [128, 256], f32, wl1_ext[:], "ld_wl1")
        wr1_t = load(const, [128, 256], f32, wr1_ext[:], "ld_wr1")
        wl2a_t = load(const, [128, 128], f32, wl2_ext[0:128, :], "ld_wl2a")
        wl2b_t = load(const, [128, 128], f32, wl2_ext[128:256, :], "ld_wl2b")
        wr2a_t = load(const, [128, 128], f32, wr2_ext[0:128, :], "ld_wr2a")
        wr2b_t = load(const, [128, 128], f32, wr2_ext[128:256, :], "ld_wr2b")
        b1_t = load(const, [128, 2], f32, b1_ext[:], "ld_b1")
        b2_t = load(const, [128, 128], f32, b2_ext[:], "ld_b2")
        jc_t = load(const, [128, 128], f32, jc_ext[:], "ld_jc")
        xT_t = load(meta, [128, NSP], f32, xT_ext[:], "ld_xT")
        esrc_t = load(meta, [128, T], i32, esrc_ext[:], "ld_esrc")
        esrc2_t = load(meta, [128, T], i32, esrc2_ext[:], "ld_esrc2")
        erel_t = load(meta, [128, T], f32, erel_ext[:], "ld_erel")
        ew_t = load(meta, [128, T], f32, ew_ext[:], "ld_ew")

        hT0 = hpool.tile([128, NSP], f32, name="hT0")
        hT1 = hpool.tile([128, NSP], f32, name="hT1")
        z_local = dram.tile([NSP, D], f32, name="z_local")
        z_full = dram.tile([M * NSP, D], f32, name="z_full", addr_space="Shared")

        def build_s(col):
            s = spool.tile([128, 128], f32, name="s")
            nc.vector.tensor_scalar(
                s[:], jc_t[:],
                erel_t[:, col:col + 1], ew_t[:, col:col + 1],
                OP.is_equal, OP.mult,
            )
            return s

        # ---------------- Layer 1 ----------------
        for w in range(WIN):
            cs, ce = w * 128, (w + 1) * 128
            p_agg = pag.tile([128, 128], f32, name="p_agg")
            for k in range(T_w):
                col = w * T_w + k
                xg = gbuf.tile([128, D], f32, name="xg")
                nc.gpsimd.indirect_dma_start(
                    out=xg[:], out_offset=None, in_=x_ext[:],
                    in_offset=bass.IndirectOffsetOnAxis(
                        ap=esrc_t[:, col:col + 1], axis=0),
                )
                s = build_s(col)
                nc.tensor.matmul(
                    out=p_agg[:], lhsT=xg[:], rhs=s[:],
                    start=(k == 0), stop=(k == T_w - 1),
                )
            aggT = work.tile([128, 128], f32, name="aggT")
            nc.vector.tensor_copy(aggT[:], p_agg[:])
            for j in range(2):
                p_h = ph.tile([128, 128], f32, name="p_h")
                nc.tensor.matmul(
                    out=p_h[:], lhsT=wl1_t[:, j * 128:(j + 1) * 128], rhs=aggT[:],
                    start=True, stop=False)
                nc.tensor.matmul(
                    out=p_h[:], lhsT=wr1_t[:, j * 128:(j + 1) * 128],
                    rhs=xT_t[:, cs:ce], start=False, stop=True)
                # exact GELU, stored unscaled: h = u * (1 + erf(u/sqrt(2)))
                # (the 0.5 is folded into W_l2/W_r2 on the host)
                u = work.tile([128, 128], f32, name="u")
                nc.scalar.activation(u[:], p_h[:], AF.Identity, bias=b1_t[:, j:j + 1])
                t_ = work.tile([128, 128], f32, name="t_")
                nc.scalar.activation(t_[:], u[:], AF.Erf, scale=SQRT_HALF)
                v = work.tile([128, 128], f32, name="v")
                nc.vector.tensor_tensor(v[:], u[:], t_[:], op=OP.mult)
                hT = hT0 if j == 0 else hT1
                nc.vector.tensor_tensor(hT[:, cs:ce], u[:], v[:], op=OP.add)
            p_z = pz.tile([128, 128], f32, name="p_z")
            nc.tensor.matmul(out=p_z[:], lhsT=hT0[:, cs:ce], rhs=wl2a_t[:],
                             start=True, stop=False)
            nc.tensor.matmul(out=p_z[:], lhsT=hT1[:, cs:ce], rhs=wl2b_t[:],
                             start=False, stop=True)
            zt = work.tile([128, 128], f32, name="zt")
            nc.scalar.activation(zt[:], p_z[:], AF.Copy)
            nc.sync.dma_start(z_local[cs:ce, :], zt[:])

        nc.gpsimd.collective_compute(
            "AllGather",
            mybir.AluOpType.bypass,
            replica_groups=[list(range(M))],
            ins=[z_local.opt()],
            outs=[z_full.opt()],
        )

        # ---------------- Layer 2 ----------------
        for w in range(WIN):
            cs, ce = w * 128, (w + 1) * 128
            p_o = po.tile([128, 128], f32, name="p_o")
            for k in range(T_w):
                col = w * T_w + k
                zg = gbuf.tile([128, D], f32, name="zg")
                nc.gpsimd.indirect_dma_start(
                    out=zg[:], out_offset=None, in_=z_full,
                    in_offset=bass.IndirectOffsetOnAxis(
                        ap=esrc2_t[:, col:col + 1], axis=0),
                )
                s = build_s(col)
                nc.tensor.matmul(
                    out=p_o[:], lhsT=s[:], rhs=zg[:],
                    start=(k == 0), stop=False,
                )
            nc.tensor.matmul(out=p_o[:], lhsT=hT0[:, cs:ce], rhs=wr2a_t[:],
                             start=False, stop=False)
            nc.tensor.matmul(out=p_o[:], lhsT=hT1[:, cs:ce], rhs=wr2b_t[:],
                             start=False, stop=True)
            ot = work.tile([128, 128], f32, name="ot")
            nc.vector.tensor_tensor(ot[:], p_o[:], b2_t[:], op=OP.add)
            rows = min(128, NS - w * 128)
            nc.sync.dma_start(out_ext[w * 128:w * 128 + rows, :], ot[:rows, :])

    nc.compile()
    return nc


def _host_prep(x, edge_index, W_l1, W_r1, b1, W_l2, W_r2, b2):
    x = np.ascontiguousarray(np.asarray(x, np.float32))
    ei = np.asarray(edge_index, np.int64)
    src, dst = ei[0], ei[1]

    cnt = np.bincount(dst, minlength=N).astype(np.float32)
    inv = 1.0 / np.maximum(cnt, 1.0)

    order = np.argsort(dst, kind="stable")
    s_src = src[order]
    s_dst = dst[order]
    s_shard = s_dst // NS
    s_loc = s_dst - s_shard * NS
    s_win = s_loc // 128
    s_rel = (s_loc % 128).astype(np.float32)
    gwin = s_shard * WIN + s_win
    counts = np.bincount(gwin, minlength=M * WIN)
    T_w = max(1, int(np.ceil(counts.max() / 128)))
    T = WIN * T_w

    gstart = np.concatenate([[0], np.cumsum(counts)[:-1]])
    pos = np.arange(E) - gstart[gwin]
    part = pos % 128
    col = s_win * T_w + pos // 128

    esrc = np.zeros((M, 128, T), np.int32)
    esrc2 = np.zeros((M, 128, T), np.int32)
    erel = np.full((M, 128, T), -1.0, np.float32)
    ew = np.zeros((M, 128, T), np.float32)
    esrc[s_shard, part, col] = s_src
    src_shard = s_src // NS
    esrc2[s_shard, part, col] = src_shard * NSP + (s_src - src_shard * NS)
    erel[s_shard, part, col] = s_rel
    ew[s_shard, part, col] = inv[s_dst]

    xT = np.zeros((M, 128, NSP), np.float32)
    for c in range(M):
        xT[c, :, :NS] = x[c * NS:(c + 1) * NS].T

    W_l1 = np.ascontiguousarray(np.asarray(W_l1, np.float32))
    W_r1 = np.ascontiguousarray(np.asarray(W_r1, np.float32))
    wl2 = np.ascontiguousarray(0.5 * np.asarray(W_l2, np.float32))
    wr2 = np.ascontiguousarray(0.5 * np.asarray(W_r2, np.float32))
    b1 = np.asarray(b1, np.float32)
    b1c = np.ascontiguousarray(np.stack([b1[:128], b1[128:]], axis=1))
    b2b = np.ascontiguousarray(
        np.tile(np.asarray(b2, np.float32)[None, :], (128, 1)))
    jc = np.ascontiguousarray(
        np.tile(np.arange(128, dtype=np.float32)[None, :], (128, 1)))

    in_maps = []
    for c in range(M):
        in_maps.append({
            "xfull": x,
            "xT": np.ascontiguousarray(xT[c]),
            "esrc": np.ascontiguousarray(esrc[c]),
            "esrc2": np.ascontiguousarray(esrc2[c]),
            "erel": np.ascontiguousarray(erel[c]),
            "ew": np.ascontiguousarray(ew[c]),
            "wl1": W_l1,
            "wr1": W_r1,
            "wl2": wl2,
            "wr2": wr2,
            "b1c": b1c,
            "b2b": b2b,
            "jc": jc,
        })
    return in_maps, T_w


def kernel(x, edge_index, W_l1, W_r1, b1, W_l2, W_r2, b2, _trace=False):
    from concourse import bass_utils

    in_maps, T_w = _host_prep(x, edge_index, W_l1, W_r1, b1, W_l2, W_r2, b2)
    if T_w not in _CACHE:
        _CACHE[T_w] = _build(T_w)
    nc = _CACHE[T_w]
    res = bass_utils.run_bass_kernel_spmd(
        nc, in_maps, core_ids=list(range(M)), trace=_trace)
    out = np.concatenate([res.results[c]["out"] for c in range(M)], axis=0)
    if _trace:
        kernel.last_exec_time_ns = res.exec_time_ns
        kernel.last_results = res
    return out



# revision 9
# speedup vs baseline: 1.3539x; 1.3539x over previous
"""GraphSAGE 2-layer (mean aggregation) on 8 TRN2 NeuronCores via Bass/Tile.

Sharding: nodes partitioned into 8 contiguous shards (6250 each); each core
owns the edges whose destination lands in its shard.  Aggregation is done on
the TensorEngine as one-hot-S matmuls over gathered source rows, with the
1/count mean applied post-aggregation.  All on-chip compute is bf16 (inputs
quantized host-side), PSUM accumulation fp32.

Gathers use the SWDGE dma_gather instruction (one instruction per ~7-window
group per table) instead of per-column indirect DMAs.  Because dma_gather
indices are int16, each gather table is split below 32768 rows: x is split in
two halves, and z is all-gathered in two window-chunks (which also lets the
first AllGather overlap the tail of layer 1).
"""

import numpy as np
import ml_dtypes

N = 50000
E = 800000
D = 128
H = 256
M = 8
NS = N // M            # 6250 nodes per shard
WIN = (NS + 127) // 128  # 49 windows of 128 node slots
NSP = WIN * 128        # 6272 padded shard size
GW = 7                 # windows per gather group
G = (WIN + GW - 1) // GW  # 7 groups
XSPLIT = 25000         # x table split (both halves < 32768)
ZW0 = 28               # windows in z-chunk 0
ZSPLIT = ZW0 * 128     # 3584 rows; chunk tables: 8*3584=28672, 8*2688=21504
ZR1 = NSP - ZSPLIT     # 2688
SQRT_HALF = 0.7071067811865476

_CACHE = {}


def _sched(ca, cb):
    """Group schedule from per-window per-table column counts.

    Returns (a_off, b_off, CAg, CBg, idxbaseA, idxbaseB, colbase) where
    offsets are within-group column offsets, CAg/CBg per-group totals,
    idxbase* the per-group index-stream bases (in slots), and colbase the
    global erel column base per group (A block then B block per group).
    """
    a_off = [0] * WIN
    b_off = [0] * WIN
    CAg = [0] * G
    CBg = [0] * G
    for g in range(G):
        acc = 0
        for w in range(g * GW, min(WIN, (g + 1) * GW)):
            a_off[w] = acc
            acc += ca[w]
        CAg[g] = acc
        acc = 0
        for w in range(g * GW, min(WIN, (g + 1) * GW)):
            b_off[w] = acc
            acc += cb[w]
        CBg[g] = acc
    idxbaseA = np.concatenate([[0], np.cumsum(CAg)[:-1]]).astype(np.int64) * 128
    idxbaseB = np.concatenate([[0], np.cumsum(CBg)[:-1]]).astype(np.int64) * 128
    colbase = np.concatenate(
        [[0], np.cumsum(np.array(CAg) + np.array(CBg))[:-1]]
    ).astype(np.int64)
    return a_off, b_off, CAg, CBg, idxbaseA, idxbaseB, colbase


def _build(meta):
    import concourse.bacc as bacc
    import concourse.tile as tile
    from concourse import bass, mybir
    from contextlib import ExitStack

    ca1, cb1, ca2, cb2, CWMAX = meta
    f32 = mybir.dt.float32
    bf16 = mybir.dt.bfloat16
    i16 = mybir.dt.int16
    AF = mybir.ActivationFunctionType
    OP = mybir.AluOpType

    s1 = _sched(ca1, cb1)
    s2 = _sched(ca2, cb2)
    LA1 = sum(s1[2]) * 128
    LB1 = sum(s1[3]) * 128
    LA2 = sum(s2[2]) * 128
    LB2 = sum(s2[3]) * 128
    CT1 = sum(s1[2]) + sum(s1[3])
    CT2 = sum(s2[2]) + sum(s2[3])
    CGMAX = max(max(s1[2]), max(s1[3]), max(s2[2]), max(s2[3]))

    nc = bacc.Bacc("TRN2", target_bir_lowering=False, debug=False,
                   dynamic_dma_scratch_size=32768)

    xbf_ext = nc.dram_tensor("xbf", [N, D], bf16, kind="ExternalInput")
    xT_ext = nc.dram_tensor("xT", [128, NSP], bf16, kind="ExternalInput")
    idxA1_ext = nc.dram_tensor("idxA1", [128, LA1 // 16], i16, kind="ExternalInput")
    idxB1_ext = nc.dram_tensor("idxB1", [128, LB1 // 16], i16, kind="ExternalInput")
    idxA2_ext = nc.dram_tensor("idxA2", [128, LA2 // 16], i16, kind="ExternalInput")
    idxB2_ext = nc.dram_tensor("idxB2", [128, LB2 // 16], i16, kind="ExternalInput")
    erel1_ext = nc.dram_tensor("erel1", [128, CT1], bf16, kind="ExternalInput")
    erel2_ext = nc.dram_tensor("erel2", [128, CT2], bf16, kind="ExternalInput")
    jcw_ext = nc.dram_tensor("jcw", [128, CWMAX * 128], bf16, kind="ExternalInput")
    invb_ext = nc.dram_tensor("invb", [128, NSP], bf16, kind="ExternalInput")
    invc_ext = nc.dram_tensor("invc", [128, WIN], f32, kind="ExternalInput")
    wl1_ext = nc.dram_tensor("wl1", [128, 256], bf16, kind="ExternalInput")
    wr1_ext = nc.dram_tensor("wr1", [128, 256], bf16, kind="ExternalInput")
    wl2_ext = nc.dram_tensor("wl2", [256, 128], bf16, kind="ExternalInput")
    wr2_ext = nc.dram_tensor("wr2", [256, 128], bf16, kind="ExternalInput")
    b1_ext = nc.dram_tensor("b1c", [128, 2], f32, kind="ExternalInput")
    b2_ext = nc.dram_tensor("b2b", [128, 128], f32, kind="ExternalInput")
    out_ext = nc.dram_tensor("out", [NS, D], f32, kind="ExternalOutput")

    with tile.TileContext(nc) as tc, ExitStack() as ctx:
        const = ctx.enter_context(tc.tile_pool(name="const", bufs=1))
        meta_p = ctx.enter_context(tc.tile_pool(name="meta", bufs=1))
        hpool = ctx.enter_context(tc.tile_pool(name="hpool", bufs=1))
        gx = ctx.enter_context(tc.tile_pool(name="gx", bufs=2))
        gy = ctx.enter_context(tc.tile_pool(name="gy", bufs=2))
        spool = ctx.enter_context(tc.tile_pool(name="spool", bufs=2))
        work = ctx.enter_context(tc.tile_pool(name="work", bufs=3))
        pag = ctx.enter_context(tc.tile_pool(name="pag", bufs=2, space="PSUM"))
        ph = ctx.enter_context(tc.tile_pool(name="ph", bufs=2, space="PSUM"))
        pz = ctx.enter_context(tc.tile_pool(name="pz", bufs=2, space="PSUM"))
        dram = ctx.enter_context(tc.tile_pool(name="dram", bufs=1, space="DRAM"))

        def load(pool, shape, dt, src, nm):
            t = pool.tile(shape, dt, name=nm)
            nc.sync.dma_start(t[:], src)
            return t

        wl1_t = load(const, [128, 256], bf16, wl1_ext[:], "ld_wl1")
        wr1_t = load(const, [128, 256], bf16, wr1_ext[:], "ld_wr1")
        wl2a_t = load(const, [128, 128], bf16, wl2_ext[0:128, :], "ld_wl2a")
        wl2b_t = load(const, [128, 128], bf16, wl2_ext[128:256, :], "ld_wl2b")
        wr2a_t = load(const, [128, 128], bf16, wr2_ext[0:128, :], "ld_wr2a")
        wr2b_t = load(const, [128, 128], bf16, wr2_ext[128:256, :], "ld_wr2b")
        b1_t = load(const, [128, 2], f32, b1_ext[:], "ld_b1")
        b2_t = load(const, [128, 128], f32, b2_ext[:], "ld_b2")
        jcw_t = load(const, [128, CWMAX * 128], bf16, jcw_ext[:], "ld_jcw")
        invc_t = load(const, [128, WIN], f32, invc_ext[:], "ld_invc")
        invb_t = load(meta_p, [128, NSP], bf16, invb_ext[:], "ld_invb")
        xT_t = load(meta_p, [128, NSP], bf16, xT_ext[:], "ld_xT")
        idxA1_t = load(meta_p, [128, LA1 // 16], i16, idxA1_ext[:], "ld_iA1")
        idxB1_t = load(meta_p, [128, LB1 // 16], i16, idxB1_ext[:], "ld_iB1")
        idxA2_t = load(meta_p, [128, LA2 // 16], i16, idxA2_ext[:], "ld_iA2")
        idxB2_t = load(meta_p, [128, LB2 // 16], i16, idxB2_ext[:], "ld_iB2")
        erel1_t = load(meta_p, [128, CT1], bf16, erel1_ext[:], "ld_er1")
        erel2_t = load(meta_p, [128, CT2], bf16, erel2_ext[:], "ld_er2")

        hT0 = hpool.tile([128, NSP], bf16, name="hT0")
        hT1 = hpool.tile([128, NSP], bf16, name="hT1")
        z_local0 = dram.tile([ZSPLIT, D], bf16, name="z_local0")
        z_local1 = dram.tile([ZR1, D], bf16, name="z_local1")
        z_full0 = dram.tile([M * ZSPLIT, D], bf16, name="z_full0",
                            addr_space="Shared")
        z_full1 = dram.tile([M * ZR1, D], bf16, name="z_full1",
                            addr_space="Shared")

        def build_s(s_t, erel_t, cbase, cw):
            # s[p, k*128+j] = (j == erel[p, cbase+k])
            if cw == 0:
                return
            nc.vector.tensor_tensor(
                s_t[:, : cw * 128].rearrange("p (a b) -> p a b", b=128),
                jcw_t[:, : cw * 128].rearrange("p (a b) -> p a b", b=128),
                erel_t[:, cbase : cbase + cw].unsqueeze(2).broadcast_to(
                    [128, cw, 128]
                ),
                op=OP.is_equal,
            )

        def gathers(g, sch, idxA_t, idxB_t, tabA, tabB, nmA, nmB):
            a_off, b_off, CAg, CBg, ibA, ibB, colbase = sch
            tiles = []
            for (Cg, idx_t, tab, base, nm) in (
                (CAg[g], idxA_t, tabA, ibA[g], nmA),
                (CBg[g], idxB_t, tabB, ibB[g], nmB),
            ):
                if Cg == 0:
                    tiles.append(None)
                    continue
                t = (gx if nm.startswith("gA") else gy).tile(
                    [128, CGMAX, 128], bf16, name=nm
                )
                # ucode caps dma_gather at 1024 indices per instruction
                GCH = 8
                for j in range(0, Cg, GCH):
                    cg = min(GCH, Cg - j)
                    b = base + j * 128
                    nc.gpsimd.dma_gather(
                        t[:, j : j + cg, :],
                        tab,
                        idx_t[:, b // 16 : (b + cg * 128) // 16],
                        num_idxs=cg * 128,
                        num_idxs_reg=cg * 128,
                        elem_size=D,
                        elem_step=D,
                    )
                tiles.append(t)
            return tiles

        # ---------------- Layer 1 ----------------
        a_off1, b_off1, CAg1, CBg1, _, _, colbase1 = s1
        for g in range(G):
            gA, gB = gathers(g, s1, idxA1_t, idxB1_t,
                             xbf_ext[0:XSPLIT, :], xbf_ext[XSPLIT:N, :],
                             "gA", "gB")
            if g == 5:
                # z chunk 0 (windows 0..27) is complete by now; all-gather it
                # while the tail of layer 1 computes.  Placed after group 5's
                # gathers so it doesn't stall the gather pipeline.
                nc.gpsimd.collective_compute(
                    "AllGather", mybir.AluOpType.bypass,
                    replica_groups=[list(range(M))],
                    ins=[z_local0.opt()], outs=[z_full0.opt()],
                )
            for w in range(g * GW, min(WIN, (g + 1) * GW)):
                cs, ce = w * 128, (w + 1) * 128
                cA, cB = ca1[w], cb1[w]
                cw = cA + cB
                s_t = spool.tile([128, CWMAX * 128], bf16, name="s")
                build_s(s_t, erel1_t, colbase1[g] + a_off1[w], cA)
                if cB:
                    nc.vector.tensor_tensor(
                        s_t[:, cA * 128 : cw * 128].rearrange(
                            "p (a b) -> p a b", b=128
                        ),
                        jcw_t[:, : cB * 128].rearrange("p (a b) -> p a b", b=128),
                        erel1_t[
                            :, colbase1[g] + CAg1[g] + b_off1[w] :
                            colbase1[g] + CAg1[g] + b_off1[w] + cB,
                        ].unsqueeze(2).broadcast_to([128, cB, 128]),
                        op=OP.is_equal,
                    )
                aggT = work.tile([128, 128], bf16, name="aggT")
                if cw:
                    p_agg = pag.tile([128, 128], f32, name="p_agg")
                    mms = [(gA, a_off1[w] + k, k) for k in range(cA)] + [
                        (gB, b_off1[w] + k, cA + k) for k in range(cB)
                    ]
                    for i, (gt, tcol, scol) in enumerate(mms):
                        nc.tensor.matmul(
                            out=p_agg[:],
                            lhsT=gt[:, tcol, :],
                            rhs=s_t[:, scol * 128 : (scol + 1) * 128],
                            start=(i == 0),
                            stop=(i == len(mms) - 1),
                        )
                    nc.vector.tensor_tensor(
                        aggT[:], p_agg[:], invb_t[:, cs:ce], op=OP.mult
                    )
                else:
                    nc.vector.memset(aggT[:], 0.0)
                for j in range(2):
                    p_h = ph.tile([128, 128], f32, name="p_h")
                    nc.tensor.matmul(
                        out=p_h[:], lhsT=wl1_t[:, j * 128 : (j + 1) * 128],
                        rhs=aggT[:], start=True, stop=False)
                    nc.tensor.matmul(
                        out=p_h[:], lhsT=wr1_t[:, j * 128 : (j + 1) * 128],
                        rhs=xT_t[:, cs:ce], start=False, stop=True)
                    # exact GELU, stored unscaled: h = u * (1 + erf(u/sqrt(2)))
                    # (the 0.5 is folded into W_l2/W_r2 on the host)
                    u = work.tile([128, 128], bf16, name="u")
                    nc.scalar.activation(u[:], p_h[:], AF.Identity,
                                         bias=b1_t[:, j : j + 1])
                    t_ = work.tile([128, 128], bf16, name="t_")
                    nc.scalar.activation(t_[:], u[:], AF.Erf, scale=SQRT_HALF)
                    v = work.tile([128, 128], bf16, name="v")
                    nc.vector.tensor_tensor(v[:], u[:], t_[:], op=OP.mult)
                    hT = hT0 if j == 0 else hT1
                    nc.vector.tensor_tensor(hT[:, cs:ce], u[:], v[:], op=OP.add)
                p_z = pz.tile([128, 128], f32, name="p_z")
                nc.tensor.matmul(out=p_z[:], lhsT=hT0[:, cs:ce], rhs=wl2a_t[:],
                                 start=True, stop=False)
                nc.tensor.matmul(out=p_z[:], lhsT=hT1[:, cs:ce], rhs=wl2b_t[:],
                                 start=False, stop=True)
                zt = work.tile([128, 128], bf16, name="zt")
                nc.scalar.activation(zt[:], p_z[:], AF.Copy)
                if w < ZW0:
                    nc.sync.dma_start(z_local0[cs : cs + 128, :], zt[:])
                else:
                    zs = (w - ZW0) * 128
                    nc.sync.dma_start(z_local1[zs : zs + 128, :], zt[:])
        nc.gpsimd.collective_compute(
            "AllGather", mybir.AluOpType.bypass,
            replica_groups=[list(range(M))],
            ins=[z_local1.opt()], outs=[z_full1.opt()],
        )

        # ---------------- Layer 2 ----------------
        a_off2, b_off2, CAg2, CBg2, _, _, colbase2 = s2
        for g in range(G):
            gA, gB = gathers(g, s2, idxA2_t, idxB2_t,
                             z_full0[:], z_full1[:], "gA", "gB")
            for w in range(g * GW, min(WIN, (g + 1) * GW)):
                cs, ce = w * 128, (w + 1) * 128
                cA, cB = ca2[w], cb2[w]
                cw = cA + cB
                s_t = spool.tile([128, CWMAX * 128], bf16, name="s")
                build_s(s_t, erel2_t, colbase2[g] + a_off2[w], cA)
                if cB:
                    nc.vector.tensor_tensor(
                        s_t[:, cA * 128 : cw * 128].rearrange(
                            "p (a b) -> p a b", b=128
                        ),
                        jcw_t[:, : cB * 128].rearrange("p (a b) -> p a b", b=128),
                        erel2_t[
                            :, colbase2[g] + CAg2[g] + b_off2[w] :
                            colbase2[g] + CAg2[g] + b_off2[w] + cB,
                        ].unsqueeze(2).broadcast_to([128, cB, 128]),
                        op=OP.is_equal,
                    )
                p_d = ph.tile([128, 128], f32, name="p_h")
                nc.tensor.matmul(out=p_d[:], lhsT=hT0[:, cs:ce], rhs=wr2a_t[:],
                                 start=True, stop=False)
                nc.tensor.matmul(out=p_d[:], lhsT=hT1[:, cs:ce], rhs=wr2b_t[:],
                                 start=False, stop=True)
                t2 = work.tile([128, 128], f32, name="t2")
                if cw:
                    p_o = pag.tile([128, 128], f32, name="p_agg")
                    mms = [(gA, a_off2[w] + k, k) for k in range(cA)] + [
                        (gB, b_off2[w] + k, cA + k) for k in range(cB)
                    ]
                    for i, (gt, tcol, scol) in enumerate(mms):
                        nc.tensor.matmul(
                            out=p_o[:],
                            lhsT=s_t[:, scol * 128 : (scol + 1) * 128],
                            rhs=gt[:, tcol, :],
                            start=(i == 0),
                            stop=(i == len(mms) - 1),
                        )
                    t1 = work.tile([128, 128], f32, name="t1")
                    nc.vector.tensor_scalar(
                        t1[:], p_o[:], invc_t[:, w : w + 1], None, OP.mult
                    )
                    nc.vector.tensor_tensor(t2[:], t1[:], p_d[:], op=OP.add)
                else:
                    nc.vector.tensor_copy(t2[:], p_d[:])
                ot = work.tile([128, 128], f32, name="ot")
                nc.vector.tensor_tensor(ot[:], t2[:], b2_t[:], op=OP.add)
                rows = min(128, NS - w * 128)
                nc.sync.dma_start(out_ext[w * 128 : w * 128 + rows, :],
                                  ot[:rows, :])

    nc.compile()
    return nc


def _host_prep(x, edge_index, W_l1, W_r1, b1, W_l2, W_r2, b2):
    bf = ml_dtypes.bfloat16
    x = np.ascontiguousarray(np.asarray(x, np.float32))
    ei = np.asarray(edge_index, np.int64)
    src, dst = ei[0], ei[1]

    cnt = np.bincount(dst, minlength=N).astype(np.float32)
    inv = (1.0 / np.maximum(cnt, 1.0)).astype(np.float32)

    c = dst // NS
    l = dst - c * NS
    w = l // 128
    rel = (l % 128).astype(np.float32)

    # per-layer table id and in-table row index
    t1 = (src >= XSPLIT).astype(np.int64)
    i1 = src - XSPLIT * t1
    c2 = src // NS
    loc = src - c2 * NS
    t2 = (loc >= ZSPLIT).astype(np.int64)
    i2 = np.where(t2 == 0, c2 * ZSPLIT + loc, c2 * ZR1 + (loc - ZSPLIT))

    def layer_pack(t, tabidx):
        key = (c * WIN + w) * 2 + t
        order = np.argsort(key, kind="stable")
        n = np.bincount(key, minlength=M * WIN * 2)
        starts = np.concatenate([[0], np.cumsum(n)[:-1]])
        pos = np.arange(E) - starts[key[order]]
        n3 = n.reshape(M, WIN, 2)
        cols = np.ceil(n3.max(axis=0) / 128.0).astype(np.int64)  # [WIN, 2]
        ca, cb = cols[:, 0], cols[:, 1]
        sch = _sched(list(ca), list(cb))
        a_off, b_off, CAg, CBg, ibA, ibB, colbase = sch
        a_off = np.asarray(a_off)
        b_off = np.asarray(b_off)
        CAg_a = np.asarray(CAg)
        LA = CAg_a.sum() * 128
        LB = np.asarray(CBg).sum() * 128
        Ctot = colbase[-1] + CAg[-1] + CBg[-1]

        so = order
        cs_, ws_, ts_ = c[so], w[so], t[so]
        g_ = ws_ // GW
        col_in = pos // 128
        part = pos % 128
        off = np.where(ts_ == 0, a_off[ws_], b_off[ws_])
        ib = np.where(ts_ == 0, ibA[g_], ibB[g_])
        gpos = ib + (off + col_in) * 128 + part
        ecol = colbase[g_] + np.where(ts_ == 0, 0, CAg_a[g_]) + off + col_in

        idxA = np.zeros((M, 16, LA // 16), np.int16)
        idxB = np.zeros((M, 16, max(LB // 16, 1)), np.int16)
        erel = np.full((M, 128, Ctot), -1.0, np.float32)
        ti = tabidx[so]
        for tt, arr in ((0, idxA), (1, idxB)):
            msk = ts_ == tt
            arr[cs_[msk], gpos[msk] % 16, gpos[msk] // 16] = ti[msk].astype(
                np.int16
            )
        erel[cs_, part, ecol] = rel[so]
        idxA = np.tile(idxA, (1, 8, 1))
        idxB = np.tile(idxB, (1, 8, 1))
        return ca, cb, idxA, idxB, erel, Ctot

    ca1, cb1, idxA1, idxB1, erel1, CT1 = layer_pack(t1, i1)
    ca2, cb2, idxA2, idxB2, erel2, CT2 = layer_pack(t2, i2)
    CWMAX = int(max((ca1 + cb1).max(), (ca2 + cb2).max()))

    xbf = x.astype(bf)
    xT = np.zeros((M, 128, NSP), np.float32)
    for cc in range(M):
        xT[cc, :, :NS] = x[cc * NS : (cc + 1) * NS].T

    invb = np.ones((M, NSP), np.float32)
    for cc in range(M):
        invb[cc, :NS] = inv[cc * NS : (cc + 1) * NS]
    invb_b = np.broadcast_to(invb[:, None, :], (M, 128, NSP))
    invc = invb.reshape(M, WIN, 128).transpose(0, 2, 1)  # [M,128,WIN]

    W_l1 = np.asarray(W_l1, np.float32).astype(bf)
    W_r1 = np.asarray(W_r1, np.float32).astype(bf)
    wl2 = (0.5 * np.asarray(W_l2, np.float32)).astype(bf)
    wr2 = (0.5 * np.asarray(W_r2, np.float32)).astype(bf)
    b1 = np.asarray(b1, np.float32)
    b1c = np.ascontiguousarray(np.stack([b1[:128], b1[128:]], axis=1))
    b2b = np.ascontiguousarray(
        np.tile(np.asarray(b2, np.float32)[None, :], (128, 1)))
    jcw = np.ascontiguousarray(
        np.tile(np.arange(128, dtype=np.float32)[None, :], (128, CWMAX))
    ).astype(bf)

    in_maps = []
    for cc in range(M):
        in_maps.append({
            "xbf": xbf,
            "xT": np.ascontiguousarray(xT[cc]).astype(bf),
            "idxA1": np.ascontiguousarray(idxA1[cc]),
            "idxB1": np.ascontiguousarray(idxB1[cc]),
            "idxA2": np.ascontiguousarray(idxA2[cc]),
            "idxB2": np.ascontiguousarray(idxB2[cc]),
            "erel1": np.ascontiguousarray(erel1[cc]).astype(bf),
            "erel2": np.ascontiguousarray(erel2[cc]).astype(bf),
            "jcw": jcw,
            "invb": np.ascontiguousarray(invb_b[cc]).astype(bf),
            "invc": np.ascontiguousarray(invc[cc]),
            "wl1": W_l1,
            "wr1": W_r1,
            "wl2": wl2,
            "wr2": wr2,
            "b1c": b1c,
            "b2b": b2b,
        })
    meta = (
        tuple(int(v) for v in ca1),
        tuple(int(v) for v in cb1),
        tuple(int(v) for v in ca2),
        tuple(int(v) for v in cb2),
        CWMAX,
    )
    return in_maps, meta


def kernel(x, edge_index, W_l1, W_r1, b1, W_l2, W_r2, b2, _trace=False):
    from concourse import bass_utils

    in_maps, meta = _host_prep(x, edge_index, W_l1, W_r1, b1, W_l2, W_r2, b2)
    if meta not in _CACHE:
        _CACHE[meta] = _build(meta)
    nc = _CACHE[meta]
    res = bass_utils.run_bass_kernel_spmd(
        nc, in_maps, core_ids=list(range(M)), trace=_trace)
    out = np.concatenate([res.results[c]["out"] for c in range(M)], axis=0)
    if _trace:
        kernel.last_exec_time_ns = res.exec_time_ns
        kernel.last_results = res
    return out


# revision 10
# speedup vs baseline: 1.4359x; 1.0605x over previous
"""GraphSAGE 2-layer (mean aggregation) on 8 TRN2 NeuronCores via Bass/Tile.

Sharding: nodes partitioned into 8 contiguous shards (6250 each); each core
owns the edges whose destination lands in its shard.  Aggregation is done on
the TensorEngine as one-hot-S matmuls over gathered source rows, with the
1/count mean applied post-aggregation.  All on-chip compute is bf16 (inputs
quantized host-side), PSUM accumulation fp32.

Gathers use the SWDGE dma_gather instruction, whose ucode costs ~8.4ns per
gathered row on the GpSimd engine — the kernel's hard bottleneck — so slots
are packed densely: each (group, table) keeps one contiguous slot stream with
no per-window column quantization.  A gather column may span two adjacent
destination windows; the S build disambiguates by encoding the second
window's rel-dst as rel+128 and comparing the boundary column against a
shifted iota (128..255).

dma_gather indices are int16 (and capped at 1024 per instruction), so each
gather table is split below 32768 rows: x in two halves, and z all-gathered
in two window-chunks (which also lets the first AllGather overlap the tail
of layer 1, and layer 2's first gathers overlap the second AllGather).
"""

import numpy as np
import ml_dtypes

N = 50000
E = 800000
D = 128
H = 256
M = 8
NS = N // M            # 6250 nodes per shard
WIN = (NS + 127) // 128  # 49 windows of 128 node slots
NSP = WIN * 128        # 6272 padded shard size
GW = 7                 # windows per gather group
G = (WIN + GW - 1) // GW  # 7 groups
XSPLIT = 25000         # x table split (both halves < 32768)
ZW0 = 28               # windows in z-chunk 0 (= 4 groups)
ZSPLIT = ZW0 * 128     # 3584 rows; chunk tables: 8*3584=28672, 8*2688=21504
ZR1 = NSP - ZSPLIT     # 2688
GCH = 8                # gather chunk: 8 columns = 1024 idxs (ucode cap)
SQRT_HALF = 0.7071067811865476

_CACHE = {}


def _sched(m):
    """Packed slot schedule for one gather table from per-window slot counts.

    Returns (o, Cg, idxbase): o[w] = slot offset of window w within its
    group's stream; Cg[g] = gather columns of group g (ceil(total/128));
    idxbase[g] = slot base of group g in the table's packed index stream.
    """
    o = [0] * WIN
    Cg = [0] * G
    for g in range(G):
        acc = 0
        for w in range(g * GW, min(WIN, (g + 1) * GW)):
            o[w] = acc
            acc += m[w]
        Cg[g] = (acc + 127) // 128
    idxbase = [0] * G
    for g in range(1, G):
        idxbase[g] = idxbase[g - 1] + Cg[g - 1] * 128
    return o, Cg, idxbase


def _layer_geom(ma, mb):
    """Full geometry for one layer: schedules, erel column bases, widths."""
    oA, CgA, ibA = _sched(ma)
    oB, CgB, ibB = _sched(mb)
    colbaseA = [0] * G
    colbaseB = [0] * G
    tot = 0
    for g in range(G):
        colbaseA[g] = tot
        colbaseB[g] = tot + CgA[g]
        tot += CgA[g] + CgB[g]
    spans = []
    for w in range(WIN):
        sa = (oA[w] + ma[w] - 1) // 128 - oA[w] // 128 + 1
        sb = (oB[w] + mb[w] - 1) // 128 - oB[w] // 128 + 1
        spans.append((sa, sb))
    return dict(o=(oA, oB), Cg=(CgA, CgB), ib=(ibA, ibB),
                colbase=(colbaseA, colbaseB), CT=tot, spans=spans)


def _build(meta):
    import concourse.bacc as bacc
    import concourse.tile as tile
    from concourse import bass, mybir
    from contextlib import ExitStack

    ma1, mb1, ma2, mb2 = [list(x) for x in meta]
    f32 = mybir.dt.float32
    bf16 = mybir.dt.bfloat16
    i16 = mybir.dt.int16
    AF = mybir.ActivationFunctionType
    OP = mybir.AluOpType

    g1 = _layer_geom(ma1, mb1)
    g2 = _layer_geom(ma2, mb2)
    L = {}
    for nm, gg, ti in (("A1", g1, 0), ("B1", g1, 1), ("A2", g2, 0),
                       ("B2", g2, 1)):
        L[nm] = (gg["ib"][ti][-1] + gg["Cg"][ti][-1] * 128)
    SW = max(sa + sb for gg in (g1, g2) for (sa, sb) in gg["spans"])
    JW = max(max(sa, sb) for gg in (g1, g2) for (sa, sb) in gg["spans"])
    CGMAX = max(max(g1["Cg"][0]), max(g1["Cg"][1]),
                max(g2["Cg"][0]), max(g2["Cg"][1]))

    nc = bacc.Bacc("TRN2", target_bir_lowering=False, debug=False,
                   dynamic_dma_scratch_size=32768)

    xbf_ext = nc.dram_tensor("xbf", [N, D], bf16, kind="ExternalInput")
    xT_ext = nc.dram_tensor("xT", [128, NSP], bf16, kind="ExternalInput")
    idx_ext = {nm: nc.dram_tensor(f"idx{nm}", [128, L[nm] // 16], i16,
                                  kind="ExternalInput") for nm in L}
    erel1_ext = nc.dram_tensor("erel1", [128, g1["CT"]], bf16,
                               kind="ExternalInput")
    erel2_ext = nc.dram_tensor("erel2", [128, g2["CT"]], bf16,
                               kind="ExternalInput")
    jcw_ext = nc.dram_tensor("jcw", [128, JW * 128], bf16,
                             kind="ExternalInput")
    jc1_ext = nc.dram_tensor("jc1", [128, 128], bf16, kind="ExternalInput")
    invb_ext = nc.dram_tensor("invb", [128, NSP], bf16, kind="ExternalInput")
    invc_ext = nc.dram_tensor("invc", [128, WIN], f32, kind="ExternalInput")
    wl1_ext = nc.dram_tensor("wl1", [128, 256], bf16, kind="ExternalInput")
    wr1_ext = nc.dram_tensor("wr1", [128, 256], bf16, kind="ExternalInput")
    wl2_ext = nc.dram_tensor("wl2", [256, 128], bf16, kind="ExternalInput")
    wr2_ext = nc.dram_tensor("wr2", [256, 128], bf16, kind="ExternalInput")
    b1_ext = nc.dram_tensor("b1c", [128, 2], f32, kind="ExternalInput")
    b2_ext = nc.dram_tensor("b2b", [128, 128], f32, kind="ExternalInput")
    out_ext = nc.dram_tensor("out", [NS, D], f32, kind="ExternalOutput")

    with tile.TileContext(nc) as tc, ExitStack() as ctx:
        const = ctx.enter_context(tc.tile_pool(name="const", bufs=1))
        meta_p = ctx.enter_context(tc.tile_pool(name="meta", bufs=1))
        hpool = ctx.enter_context(tc.tile_pool(name="hpool", bufs=1))
        gx = ctx.enter_context(tc.tile_pool(name="gx", bufs=2))
        gy = ctx.enter_context(tc.tile_pool(name="gy", bufs=2))
        spool = ctx.enter_context(tc.tile_pool(name="spool", bufs=2))
        work = ctx.enter_context(tc.tile_pool(name="work", bufs=3))
        pag = ctx.enter_context(tc.tile_pool(name="pag", bufs=2, space="PSUM"))
        ph = ctx.enter_context(tc.tile_pool(name="ph", bufs=2, space="PSUM"))
        pz = ctx.enter_context(tc.tile_pool(name="pz", bufs=2, space="PSUM"))
        dram = ctx.enter_context(tc.tile_pool(name="dram", bufs=1,
                                              space="DRAM"))

        def load(pool, shape, dt, src, nm):
            t = pool.tile(shape, dt, name=nm)
            nc.sync.dma_start(t[:], src)
            return t

        wl1_t = load(const, [128, 256], bf16, wl1_ext[:], "ld_wl1")
        wr1_t = load(const, [128, 256], bf16, wr1_ext[:], "ld_wr1")
        wl2a_t = load(const, [128, 128], bf16, wl2_ext[0:128, :], "ld_wl2a")
        wl2b_t = load(const, [128, 128], bf16, wl2_ext[128:256, :], "ld_wl2b")
        wr2a_t = load(const, [128, 128], bf16, wr2_ext[0:128, :], "ld_wr2a")
        wr2b_t = load(const, [128, 128], bf16, wr2_ext[128:256, :], "ld_wr2b")
        b1_t = load(const, [128, 2], f32, b1_ext[:], "ld_b1")
        b2_t = load(const, [128, 128], f32, b2_ext[:], "ld_b2")
        jcw_t = load(const, [128, JW * 128], bf16, jcw_ext[:], "ld_jcw")
        jc1_t = load(const, [128, 128], bf16, jc1_ext[:], "ld_jc1")
        invc_t = load(const, [128, WIN], f32, invc_ext[:], "ld_invc")
        invb_t = load(meta_p, [128, NSP], bf16, invb_ext[:], "ld_invb")
        xT_t = load(meta_p, [128, NSP], bf16, xT_ext[:], "ld_xT")
        idx_t = {nm: load(meta_p, [128, L[nm] // 16], i16, idx_ext[nm][:],
                          f"ld_i{nm}") for nm in L}
        erel1_t = load(meta_p, [128, g1["CT"]], bf16, erel1_ext[:], "ld_er1")
        erel2_t = load(meta_p, [128, g2["CT"]], bf16, erel2_ext[:], "ld_er2")

        hT0 = hpool.tile([128, NSP], bf16, name="hT0")
        hT1 = hpool.tile([128, NSP], bf16, name="hT1")
        z_local0 = dram.tile([ZSPLIT, D], bf16, name="z_local0")
        z_local1 = dram.tile([ZR1, D], bf16, name="z_local1")
        z_full0 = dram.tile([M * ZSPLIT, D], bf16, name="z_full0",
                            addr_space="Shared")
        z_full1 = dram.tile([M * ZR1, D], bf16, name="z_full1",
                            addr_space="Shared")

        def gathers(g, gg, itA, itB, tabA, tabB):
            tiles = []
            for ti, (it, tab, pool) in enumerate(
                ((itA, tabA, gx), (itB, tabB, gy))
            ):
                Cg = gg["Cg"][ti][g]
                base = gg["ib"][ti][g]
                t = pool.tile([128, CGMAX, 128], bf16, name="g")
                for j in range(0, Cg, GCH):
                    cg = min(GCH, Cg - j)
                    b = base + j * 128
                    nc.gpsimd.dma_gather(
                        t[:, j : j + cg, :], tab,
                        it[:, b // 16 : (b + cg * 128) // 16],
                        num_idxs=cg * 128, num_idxs_reg=cg * 128,
                        elem_size=D, elem_step=D,
                    )
                tiles.append(t)
            return tiles

        def build_s_and_cols(s_t, w, g, gg, erel_t, gA, gB):
            """Emit S builds for window w; return [(gtile, col, scol)] list."""
            cols = []
            so = 0
            for ti, gt in ((0, gA), (1, gB)):
                o = gg["o"][ti][w]
                m = (ma1 if gg is g1 else ma2)  # unused; spans recomputed
                span = gg["spans"][w][ti]
                c0 = o // 128
                cbase = gg["colbase"][ti][g]
                bnd = 1 if (o % 128) else 0
                if bnd:
                    nc.vector.tensor_tensor(
                        s_t[:, so * 128 : (so + 1) * 128],
                        jc1_t[:],
                        erel_t[:, cbase + c0 : cbase + c0 + 1]
                        .broadcast_to([128, 128]),
                        op=OP.is_equal,
                    )
                rest = span - bnd
                if rest:
                    nc.vector.tensor_tensor(
                        s_t[:, (so + bnd) * 128 : (so + span) * 128]
                        .rearrange("p (a b) -> p a b", b=128),
                        jcw_t[:, : rest * 128]
                        .rearrange("p (a b) -> p a b", b=128),
                        erel_t[:, cbase + c0 + bnd : cbase + c0 + span]
                        .unsqueeze(2).broadcast_to([128, rest, 128]),
                        op=OP.is_equal,
                    )
                for k in range(span):
                    cols.append((gt, c0 + k, so + k))
                so += span
            return cols

        # ---------------- Layer 1 ----------------
        for g in range(G):
            gA, gB = gathers(g, g1, idx_t["A1"], idx_t["B1"],
                             xbf_ext[0:XSPLIT, :], xbf_ext[XSPLIT:N, :])
            if g == 5:
                # z chunk 0 (windows 0..27) is complete by now; all-gather it
                # while the tail of layer 1 computes.  Placed after group 5's
                # gathers so it doesn't stall the gather pipeline.
                nc.gpsimd.collective_compute(
                    "AllGather", mybir.AluOpType.bypass,
                    replica_groups=[list(range(M))],
                    ins=[z_local0.opt()], outs=[z_full0.opt()],
                )
            for w in range(g * GW, min(WIN, (g + 1) * GW)):
                cs, ce = w * 128, (w + 1) * 128
                s_t = spool.tile([128, SW * 128], bf16, name="s")
                mms = build_s_and_cols(s_t, w, g, g1, erel1_t, gA, gB)
                p_agg = pag.tile([128, 128], f32, name="p_agg")
                for i, (gt, tcol, scol) in enumerate(mms):
                    nc.tensor.matmul(
                        out=p_agg[:], lhsT=gt[:, tcol, :],
                        rhs=s_t[:, scol * 128 : (scol + 1) * 128],
                        start=(i == 0), stop=(i == len(mms) - 1),
                    )
                aggT = work.tile([128, 128], bf16, name="aggT")
                nc.vector.tensor_tensor(
                    aggT[:], p_agg[:], invb_t[:, cs:ce], op=OP.mult)
                for j in range(2):
                    p_h = ph.tile([128, 128], f32, name="p_h")
                    nc.tensor.matmul(
                        out=p_h[:], lhsT=wl1_t[:, j * 128 : (j + 1) * 128],
                        rhs=aggT[:], start=True, stop=False)
                    nc.tensor.matmul(
                        out=p_h[:], lhsT=wr1_t[:, j * 128 : (j + 1) * 128],
                        rhs=xT_t[:, cs:ce], start=False, stop=True)
                    # exact GELU, stored unscaled: h = u * (1 + erf(u/sqrt2))
                    # (the 0.5 is folded into W_l2/W_r2 on the host)
                    u = work.tile([128, 128], bf16, name="u")
                    nc.scalar.activation(u[:], p_h[:], AF.Identity,
                                         bias=b1_t[:, j : j + 1])
                    t_ = work.tile([128, 128], bf16, name="t_")
                    nc.scalar.activation(t_[:], u[:], AF.Erf, scale=SQRT_HALF)
                    v = work.tile([128, 128], bf16, name="v")
                    nc.vector.tensor_tensor(v[:], u[:], t_[:], op=OP.mult)
                    hT = hT0 if j == 0 else hT1
                    nc.vector.tensor_tensor(hT[:, cs:ce], u[:], v[:],
                                            op=OP.add)
                p_z = pz.tile([128, 128], f32, name="p_z")
                nc.tensor.matmul(out=p_z[:], lhsT=hT0[:, cs:ce],
                                 rhs=wl2a_t[:], start=True, stop=False)
                nc.tensor.matmul(out=p_z[:], lhsT=hT1[:, cs:ce],
                                 rhs=wl2b_t[:], start=False, stop=True)
                zt = work.tile([128, 128], bf16, name="zt")
                nc.scalar.activation(zt[:], p_z[:], AF.Copy)
                if w < ZW0:
                    nc.sync.dma_start(z_local0[cs : cs + 128, :], zt[:])
                else:
                    zs = (w - ZW0) * 128
                    nc.sync.dma_start(z_local1[zs : zs + 128, :], zt[:])

        # ---------------- Layer 2 ----------------
        for g in range(G):
            # table C (z chunk 0) only needs the first AllGather; issue its
            # gathers before the second AllGather so they overlap it.
            CgC = g2["Cg"][0][g]
            baseC = g2["ib"][0][g]
            gA = gx.tile([128, CGMAX, 128], bf16, name="g")
            for j in range(0, CgC, GCH):
                cg = min(GCH, CgC - j)
                b = baseC + j * 128
                nc.gpsimd.dma_gather(
                    gA[:, j : j + cg, :], z_full0[:],
                    idx_t["A2"][:, b // 16 : (b + cg * 128) // 16],
                    num_idxs=cg * 128, num_idxs_reg=cg * 128,
                    elem_size=D, elem_step=D,
                )
            if g == 0:
                nc.gpsimd.collective_compute(
                    "AllGather", mybir.AluOpType.bypass,
                    replica_groups=[list(range(M))],
                    ins=[z_local1.opt()], outs=[z_full1.opt()],
                )
            CgD = g2["Cg"][1][g]
            baseD = g2["ib"][1][g]
            gB = gy.tile([128, CGMAX, 128], bf16, name="g")
            for j in range(0, CgD, GCH):
                cg = min(GCH, CgD - j)
                b = baseD + j * 128
                nc.gpsimd.dma_gather(
                    gB[:, j : j + cg, :], z_full1[:],
                    idx_t["B2"][:, b // 16 : (b + cg * 128) // 16],
                    num_idxs=cg * 128, num_idxs_reg=cg * 128,
                    elem_size=D, elem_step=D,
                )
            for w in range(g * GW, min(WIN, (g + 1) * GW)):
                cs, ce = w * 128, (w + 1) * 128
                s_t = spool.tile([128, SW * 128], bf16, name="s")
                mms = build_s_and_cols(s_t, w, g, g2, erel2_t, gA, gB)
                p_d = ph.tile([128, 128], f32, name="p_h")
                nc.tensor.matmul(out=p_d[:], lhsT=hT0[:, cs:ce],
                                 rhs=wr2a_t[:], start=True, stop=False)
                nc.tensor.matmul(out=p_d[:], lhsT=hT1[:, cs:ce],
                                 rhs=wr2b_t[:], start=False, stop=True)
                p_o = pag.tile([128, 128], f32, name="p_agg")
                for i, (gt, tcol, scol) in enumerate(mms):
                    nc.tensor.matmul(
                        out=p_o[:],
                        lhsT=s_t[:, scol * 128 : (scol + 1) * 128],
                        rhs=gt[:, tcol, :],
                        start=(i == 0), stop=(i == len(mms) - 1),
                    )
                t1 = work.tile([128, 128], f32, name="t1")
                nc.vector.tensor_scalar(
                    t1[:], p_o[:], invc_t[:, w : w + 1], None, OP.mult)
                t2 = work.tile([128, 128], f32, name="t2")
                nc.vector.tensor_tensor(t2[:], t1[:], p_d[:], op=OP.add)
                ot = work.tile([128, 128], f32, name="ot")
                nc.vector.tensor_tensor(ot[:], t2[:], b2_t[:], op=OP.add)
                rows = min(128, NS - w * 128)
                nc.sync.dma_start(out_ext[w * 128 : w * 128 + rows, :],
                                  ot[:rows, :])

    nc.compile()
    return nc


def _host_prep(x, edge_index, W_l1, W_r1, b1, W_l2, W_r2, b2):
    bf = ml_dtypes.bfloat16
    x = np.ascontiguousarray(np.asarray(x, np.float32))
    ei = np.asarray(edge_index, np.int64)
    src, dst = ei[0], ei[1]

    cnt = np.bincount(dst, minlength=N).astype(np.float32)
    inv = (1.0 / np.maximum(cnt, 1.0)).astype(np.float32)

    c = dst // NS
    l = dst - c * NS
    w = l // 128
    rel = (l % 128).astype(np.int64)

    t1 = (src >= XSPLIT).astype(np.int64)
    i1 = src - XSPLIT * t1
    c2 = src // NS
    loc = src - c2 * NS
    t2 = (loc >= ZSPLIT).astype(np.int64)
    i2 = np.where(t2 == 0, c2 * ZSPLIT + loc, c2 * ZR1 + (loc - ZSPLIT))

    def layer_pack(t, tabidx):
        key = (c * WIN + w) * 2 + t
        order = np.argsort(key, kind="stable")
        n = np.bincount(key, minlength=M * WIN * 2)
        starts = np.concatenate([[0], np.cumsum(n)[:-1]])
        pos = np.arange(E) - starts[key[order]]
        n3 = n.reshape(M, WIN, 2)
        m = n3.max(axis=0)  # [WIN, 2] slot counts (max over cores)
        assert (m >= 128).all(), "window-table with <128 edges unsupported"
        ma, mb = [int(v) for v in m[:, 0]], [int(v) for v in m[:, 1]]
        gg = _layer_geom(ma, mb)
        oA = np.asarray(gg["o"][0])
        oB = np.asarray(gg["o"][1])
        ibA = np.asarray(gg["ib"][0])
        ibB = np.asarray(gg["ib"][1])
        cbA = np.asarray(gg["colbase"][0])
        cbB = np.asarray(gg["colbase"][1])
        LA = int(ibA[-1] + gg["Cg"][0][-1] * 128)
        LB = int(ibB[-1] + gg["Cg"][1][-1] * 128)

        so = order
        cs_, ws_, ts_ = c[so], w[so], t[so]
        g_ = ws_ // GW
        o = np.where(ts_ == 0, oA[ws_], oB[ws_])
        s_grp = o + pos                      # slot within group stream
        part = s_grp % 128
        col_l = s_grp // 128                 # column within group stream
        ib = np.where(ts_ == 0, ibA[g_], ibB[g_])
        gpos = ib + s_grp                    # slot within table idx stream
        cb = np.where(ts_ == 0, cbA[g_], cbB[g_])
        ecol = cb + col_l
        enc = rel[so] + 128 * (col_l * 128 < o)

        idxA = np.zeros((M, 16, LA // 16), np.int16)
        idxB = np.zeros((M, 16, LB // 16), np.int16)
        CT = gg["CT"]
        erel = np.full((M, 128, CT), -1.0, np.float32)
        ti_ = tabidx[so]
        for tt, arr in ((0, idxA), (1, idxB)):
            msk = ts_ == tt
            arr[cs_[msk], gpos[msk] % 16, gpos[msk] // 16] = ti_[msk].astype(
                np.int16)
        erel[cs_, part, ecol] = enc
        idxA = np.tile(idxA, (1, 8, 1))
        idxB = np.tile(idxB, (1, 8, 1))
        return ma, mb, idxA, idxB, erel

    ma1, mb1, idxA1, idxB1, erel1 = layer_pack(t1, i1)
    ma2, mb2, idxA2, idxB2, erel2 = layer_pack(t2, i2)
    g1 = _layer_geom(ma1, mb1)
    g2 = _layer_geom(ma2, mb2)
    JW = max(max(sa, sb) for gg in (g1, g2) for (sa, sb) in gg["spans"])

    xbf = x.astype(bf)
    xT = np.zeros((M, 128, NSP), np.float32)
    for cc in range(M):
        xT[cc, :, :NS] = x[cc * NS : (cc + 1) * NS].T

    invb = np.ones((M, NSP), np.float32)
    for cc in range(M):
        invb[cc, :NS] = inv[cc * NS : (cc + 1) * NS]
    invb_b = np.broadcast_to(invb[:, None, :], (M, 128, NSP))
    invc = invb.reshape(M, WIN, 128).transpose(0, 2, 1)  # [M,128,WIN]

    W_l1 = np.asarray(W_l1, np.float32).astype(bf)
    W_r1 = np.asarray(W_r1, np.float32).astype(bf)
    wl2 = (0.5 * np.asarray(W_l2, np.float32)).astype(bf)
    wr2 = (0.5 * np.asarray(W_r2, np.float32)).astype(bf)
    b1 = np.asarray(b1, np.float32)
    b1c = np.ascontiguousarray(np.stack([b1[:128], b1[128:]], axis=1))
    b2b = np.ascontiguousarray(
        np.tile(np.asarray(b2, np.float32)[None, :], (128, 1)))
    jcw = np.ascontiguousarray(
        np.tile(np.arange(128, dtype=np.float32)[None, :], (128, JW))
    ).astype(bf)
    jc1 = np.ascontiguousarray(
        np.tile(np.arange(128, 256, dtype=np.float32)[None, :], (128, 1))
    ).astype(bf)

    in_maps = []
    for cc in range(M):
        in_maps.append({
            "xbf": xbf,
            "xT": np.ascontiguousarray(xT[cc]).astype(bf),
            "idxA1": np.ascontiguousarray(idxA1[cc]),
            "idxB1": np.ascontiguousarray(idxB1[cc]),
            "idxA2": np.ascontiguousarray(idxA2[cc]),
            "idxB2": np.ascontiguousarray(idxB2[cc]),
            "erel1": np.ascontiguousarray(erel1[cc]).astype(bf),
            "erel2": np.ascontiguousarray(erel2[cc]).astype(bf),
            "jcw": jcw,
            "jc1": jc1,
            "invb": np.ascontiguousarray(invb_b[cc]).astype(bf),
            "invc": np.ascontiguousarray(invc[cc]),
            "wl1": W_l1,
            "wr1": W_r1,
            "wl2": wl2,
            "wr2": wr2,
            "b1c": b1c,
            "b2b": b2b,
        })
    meta = (tuple(ma1), tuple(mb1), tuple(ma2), tuple(mb2))
    return in_maps, meta


def kernel(x, edge_index, W_l1, W_r1, b1, W_l2, W_r2, b2, _trace=False):
    from concourse import bass_utils

    in_maps, meta = _host_prep(x, edge_index, W_l1, W_r1, b1, W_l2, W_r2, b2)
    if meta not in _CACHE:
        _CACHE[meta] = _build(meta)
    nc = _CACHE[meta]
    res = bass_utils.run_bass_kernel_spmd(
        nc, in_maps, core_ids=list(range(M)), trace=_trace)
    out = np.concatenate([res.results[c]["out"] for c in range(M)], axis=0)
    if _trace:
        kernel.last_exec_time_ns = res.exec_time_ns
        kernel.last_results = res
    return out


# revision 11
# speedup vs baseline: 1.5000x; 1.0446x over previous
"""GraphSAGE 2-layer (mean aggregation) on 8 TRN2 NeuronCores via Bass/Tile.

Sharding: nodes partitioned into 8 contiguous shards (6250 each); each core
owns the edges whose destination lands in its shard.  Aggregation is done on
the TensorEngine as one-hot-S matmuls over gathered source rows, with the
1/count mean applied post-aggregation.  All on-chip compute is bf16 (inputs
quantized host-side), PSUM accumulation fp32.

Gathers use the SWDGE dma_gather instruction, whose ucode costs ~8.4ns per
gathered row on the GpSimd engine — the kernel's hard bottleneck — so slots
are packed densely: each (group, table) keeps one contiguous slot stream with
no per-window column quantization.  A gather column may span two adjacent
destination windows; the S build disambiguates by encoding the second
window's rel-dst as rel+128 and comparing the boundary column against a
shifted iota (128..255).

dma_gather indices are int16 (and capped at 1024 per instruction), so each
gather table is split below 32768 rows: x in two halves, and z all-gathered
in two window-chunks (which also lets the first AllGather overlap the tail
of layer 1, and layer 2's first gathers overlap the second AllGather).
"""

import numpy as np
import ml_dtypes

N = 50000
E = 800000
D = 128
H = 256
M = 8
NS = N // M            # 6250 nodes per shard
WIN = (NS + 127) // 128  # 49 windows of 128 node slots
NSP = WIN * 128        # 6272 padded shard size
GW = 7                 # windows per gather group
G = (WIN + GW - 1) // GW  # 7 groups
XSPLIT = 25000         # x table split (both halves < 32768)
ZW0 = 28               # windows in z-chunk 0 (= 4 groups)
ZSPLIT = ZW0 * 128     # 3584 rows; chunk tables: 8*3584=28672, 8*2688=21504
ZR1 = NSP - ZSPLIT     # 2688
GCH = 8                # gather chunk: 8 columns = 1024 idxs (ucode cap)
SQRT_HALF = 0.7071067811865476

_CACHE = {}


def _sched(m):
    """Packed slot schedule for one gather table from per-window slot counts.

    Returns (o, Cg, idxbase): o[w] = slot offset of window w within its
    group's stream; Cg[g] = gather columns of group g (ceil(total/128));
    idxbase[g] = slot base of group g in the table's packed index stream.
    """
    o = [0] * WIN
    Cg = [0] * G
    for g in range(G):
        acc = 0
        for w in range(g * GW, min(WIN, (g + 1) * GW)):
            o[w] = acc
            acc += m[w]
        Cg[g] = (acc + 127) // 128
    idxbase = [0] * G
    for g in range(1, G):
        idxbase[g] = idxbase[g - 1] + Cg[g - 1] * 128
    return o, Cg, idxbase


def _layer_geom(ma, mb):
    """Full geometry for one layer: schedules, erel column bases, widths."""
    oA, CgA, ibA = _sched(ma)
    oB, CgB, ibB = _sched(mb)
    colbaseA = [0] * G
    colbaseB = [0] * G
    tot = 0
    for g in range(G):
        colbaseA[g] = tot
        colbaseB[g] = tot + CgA[g]
        tot += CgA[g] + CgB[g]
    spans = []
    for w in range(WIN):
        sa = (oA[w] + ma[w] - 1) // 128 - oA[w] // 128 + 1
        sb = (oB[w] + mb[w] - 1) // 128 - oB[w] // 128 + 1
        spans.append((sa, sb))
    return dict(o=(oA, oB), Cg=(CgA, CgB), ib=(ibA, ibB),
                colbase=(colbaseA, colbaseB), CT=tot, spans=spans)


def _build(meta):
    import concourse.bacc as bacc
    import concourse.tile as tile
    from concourse import bass, mybir
    from contextlib import ExitStack

    ma1, mb1, ma2, mb2 = [list(x) for x in meta]
    f32 = mybir.dt.float32
    bf16 = mybir.dt.bfloat16
    i16 = mybir.dt.int16
    AF = mybir.ActivationFunctionType
    OP = mybir.AluOpType

    g1 = _layer_geom(ma1, mb1)
    g2 = _layer_geom(ma2, mb2)
    L = {}
    for nm, gg, ti in (("A1", g1, 0), ("B1", g1, 1), ("A2", g2, 0),
                       ("B2", g2, 1)):
        L[nm] = (gg["ib"][ti][-1] + gg["Cg"][ti][-1] * 128)
    SW = max(sa + sb for gg in (g1, g2) for (sa, sb) in gg["spans"])
    JW = max(max(sa, sb) for gg in (g1, g2) for (sa, sb) in gg["spans"])
    CGMAX = max(max(g1["Cg"][0]), max(g1["Cg"][1]),
                max(g2["Cg"][0]), max(g2["Cg"][1]))

    nc = bacc.Bacc("TRN2", target_bir_lowering=False, debug=False,
                   dynamic_dma_scratch_size=32768)

    xbf_ext = nc.dram_tensor("xbf", [N, D], bf16, kind="ExternalInput")
    xT_ext = nc.dram_tensor("xT", [128, NSP], bf16, kind="ExternalInput")
    idx_ext = {nm: nc.dram_tensor(f"idx{nm}", [128, L[nm] // 16], i16,
                                  kind="ExternalInput") for nm in L}
    erel1_ext = nc.dram_tensor("erel1", [128, g1["CT"]], bf16,
                               kind="ExternalInput")
    erel2_ext = nc.dram_tensor("erel2", [128, g2["CT"]], bf16,
                               kind="ExternalInput")
    jcb_ext = nc.dram_tensor("jcb", [128, (JW + 1) * 128], bf16,
                             kind="ExternalInput")
    invb_ext = nc.dram_tensor("invb", [128, NSP], bf16, kind="ExternalInput")
    invc_ext = nc.dram_tensor("invc", [128, WIN], f32, kind="ExternalInput")
    wl1_ext = nc.dram_tensor("wl1", [128, 256], bf16, kind="ExternalInput")
    wr1_ext = nc.dram_tensor("wr1", [128, 256], bf16, kind="ExternalInput")
    wl2_ext = nc.dram_tensor("wl2", [256, 128], bf16, kind="ExternalInput")
    wr2_ext = nc.dram_tensor("wr2", [256, 128], bf16, kind="ExternalInput")
    b1_ext = nc.dram_tensor("b1c", [128, 2], f32, kind="ExternalInput")
    b2_ext = nc.dram_tensor("b2b", [128, 128], f32, kind="ExternalInput")
    out_ext = nc.dram_tensor("out", [NS, D], f32, kind="ExternalOutput")

    with tile.TileContext(nc) as tc, ExitStack() as ctx:
        const = ctx.enter_context(tc.tile_pool(name="const", bufs=1))
        meta_p = ctx.enter_context(tc.tile_pool(name="meta", bufs=1))
        hpool = ctx.enter_context(tc.tile_pool(name="hpool", bufs=1))
        gx = ctx.enter_context(tc.tile_pool(name="gx", bufs=2))
        gy = ctx.enter_context(tc.tile_pool(name="gy", bufs=2))
        spool = ctx.enter_context(tc.tile_pool(name="spool", bufs=2))
        work = ctx.enter_context(tc.tile_pool(name="work", bufs=3))
        pag = ctx.enter_context(tc.tile_pool(name="pag", bufs=2, space="PSUM"))
        ph = ctx.enter_context(tc.tile_pool(name="ph", bufs=2, space="PSUM"))
        pz = ctx.enter_context(tc.tile_pool(name="pz", bufs=2, space="PSUM"))
        dram = ctx.enter_context(tc.tile_pool(name="dram", bufs=1,
                                              space="DRAM"))

        def load(pool, shape, dt, src, nm):
            t = pool.tile(shape, dt, name=nm)
            nc.sync.dma_start(t[:], src)
            return t

        idx_t = {nm: load(meta_p, [128, L[nm] // 16], i16, idx_ext[nm][:],
                          f"ld_i{nm}") for nm in L}
        erel1_t = load(meta_p, [128, g1["CT"]], bf16, erel1_ext[:], "ld_er1")
        erel2_t = load(meta_p, [128, g2["CT"]], bf16, erel2_ext[:], "ld_er2")
        wl1_t = load(const, [128, 256], bf16, wl1_ext[:], "ld_wl1")
        wr1_t = load(const, [128, 256], bf16, wr1_ext[:], "ld_wr1")
        wl2a_t = load(const, [128, 128], bf16, wl2_ext[0:128, :], "ld_wl2a")
        wl2b_t = load(const, [128, 128], bf16, wl2_ext[128:256, :], "ld_wl2b")
        wr2a_t = load(const, [128, 128], bf16, wr2_ext[0:128, :], "ld_wr2a")
        wr2b_t = load(const, [128, 128], bf16, wr2_ext[128:256, :], "ld_wr2b")
        b1_t = load(const, [128, 2], f32, b1_ext[:], "ld_b1")
        b2_t = load(const, [128, 128], f32, b2_ext[:], "ld_b2")
        jcb_t = load(const, [128, (JW + 1) * 128], bf16, jcb_ext[:], "ld_jcb")
        invc_t = load(const, [128, WIN], f32, invc_ext[:], "ld_invc")
        invb_t = load(meta_p, [128, NSP], bf16, invb_ext[:], "ld_invb")
        xT_t = load(meta_p, [128, NSP], bf16, xT_ext[:], "ld_xT")

        hT0 = hpool.tile([128, NSP], bf16, name="hT0")
        hT1 = hpool.tile([128, NSP], bf16, name="hT1")
        z_local0 = dram.tile([ZSPLIT, D], bf16, name="z_local0")
        z_local1 = dram.tile([ZR1, D], bf16, name="z_local1")
        z_full0 = dram.tile([M * ZSPLIT, D], bf16, name="z_full0",
                            addr_space="Shared")
        z_full1 = dram.tile([M * ZR1, D], bf16, name="z_full1",
                            addr_space="Shared")

        def gathers(g, gg, itA, itB, tabA, tabB):
            tiles = []
            for ti, (it, tab, pool) in enumerate(
                ((itA, tabA, gx), (itB, tabB, gy))
            ):
                Cg = gg["Cg"][ti][g]
                base = gg["ib"][ti][g]
                t = pool.tile([128, CGMAX, 128], bf16, name="g")
                for j in range(0, Cg, GCH):
                    cg = min(GCH, Cg - j)
                    b = base + j * 128
                    nc.gpsimd.dma_gather(
                        t[:, j : j + cg, :], tab,
                        it[:, b // 16 : (b + cg * 128) // 16],
                        num_idxs=cg * 128, num_idxs_reg=cg * 128,
                        elem_size=D, elem_step=D,
                    )
                tiles.append(t)
            return tiles

        def build_s_and_cols(s_t, w, g, gg, erel_t, gA, gB):
            """Emit S builds for window w; return [(gtile, col, scol)] list."""
            cols = []
            so = 0
            for ti, gt in ((0, gA), (1, gB)):
                o = gg["o"][ti][w]
                span = gg["spans"][w][ti]
                c0 = o // 128
                cbase = gg["colbase"][ti][g]
                # jcb = [128..255 | 0..127 0..127 ...]: a boundary column
                # (window starts mid-column; its slots encoded rel+128)
                # compares against the shifted iota, the rest against 0..127.
                j0 = 0 if (o % 128) else 128
                nc.vector.tensor_tensor(
                    s_t[:, so * 128 : (so + span) * 128]
                    .rearrange("p (a b) -> p a b", b=128),
                    jcb_t[:, j0 : j0 + span * 128]
                    .rearrange("p (a b) -> p a b", b=128),
                    erel_t[:, cbase + c0 : cbase + c0 + span]
                    .unsqueeze(2).broadcast_to([128, span, 128]),
                    op=OP.is_equal,
                )
                for k in range(span):
                    cols.append((gt, c0 + k, so + k))
                so += span
            return cols

        # ---------------- Layer 1 ----------------
        for g in range(G):
            gA, gB = gathers(g, g1, idx_t["A1"], idx_t["B1"],
                             xbf_ext[0:XSPLIT, :], xbf_ext[XSPLIT:N, :])
            if g == 5:
                # z chunk 0 (windows 0..27) is complete by now; all-gather it
                # while the tail of layer 1 computes.  Placed after group 5's
                # gathers so it doesn't stall the gather pipeline.
                nc.gpsimd.collective_compute(
                    "AllGather", mybir.AluOpType.bypass,
                    replica_groups=[list(range(M))],
                    ins=[z_local0.opt()], outs=[z_full0.opt()],
                )
            for w in range(g * GW, min(WIN, (g + 1) * GW)):
                cs, ce = w * 128, (w + 1) * 128
                s_t = spool.tile([128, SW * 128], bf16, name="s")
                mms = build_s_and_cols(s_t, w, g, g1, erel1_t, gA, gB)
                p_agg = pag.tile([128, 128], f32, name="p_agg")
                for i, (gt, tcol, scol) in enumerate(mms):
                    nc.tensor.matmul(
                        out=p_agg[:], lhsT=gt[:, tcol, :],
                        rhs=s_t[:, scol * 128 : (scol + 1) * 128],
                        start=(i == 0), stop=(i == len(mms) - 1),
                    )
                aggT = work.tile([128, 128], bf16, name="aggT")
                nc.vector.tensor_tensor(
                    aggT[:], p_agg[:], invb_t[:, cs:ce], op=OP.mult)
                for j in range(2):
                    p_h = ph.tile([128, 128], f32, name="p_h")
                    nc.tensor.matmul(
                        out=p_h[:], lhsT=wl1_t[:, j * 128 : (j + 1) * 128],
                        rhs=aggT[:], start=True, stop=False)
                    nc.tensor.matmul(
                        out=p_h[:], lhsT=wr1_t[:, j * 128 : (j + 1) * 128],
                        rhs=xT_t[:, cs:ce], start=False, stop=True)
                    hT = hT0 if j == 0 else hT1
                    nc.scalar.activation(hT[:, cs:ce], p_h[:], AF.Gelu,
                                         bias=b1_t[:, j : j + 1])
                p_z = pz.tile([128, 128], f32, name="p_z")
                nc.tensor.matmul(out=p_z[:], lhsT=hT0[:, cs:ce],
                                 rhs=wl2a_t[:], start=True, stop=False)
                nc.tensor.matmul(out=p_z[:], lhsT=hT1[:, cs:ce],
                                 rhs=wl2b_t[:], start=False, stop=True)
                zt = work.tile([128, 128], bf16, name="zt")
                nc.scalar.activation(zt[:], p_z[:], AF.Copy)
                if w < ZW0:
                    nc.sync.dma_start(z_local0[cs : cs + 128, :], zt[:])
                else:
                    zs = (w - ZW0) * 128
                    nc.sync.dma_start(z_local1[zs : zs + 128, :], zt[:])

        # ---------------- Layer 2 ----------------
        for g in range(G):
            # table C (z chunk 0) only needs the first AllGather; issue its
            # gathers before the second AllGather so they overlap it.
            CgC = g2["Cg"][0][g]
            baseC = g2["ib"][0][g]
            gA = gx.tile([128, CGMAX, 128], bf16, name="g")
            for j in range(0, CgC, GCH):
                cg = min(GCH, CgC - j)
                b = baseC + j * 128
                nc.gpsimd.dma_gather(
                    gA[:, j : j + cg, :], z_full0[:],
                    idx_t["A2"][:, b // 16 : (b + cg * 128) // 16],
                    num_idxs=cg * 128, num_idxs_reg=cg * 128,
                    elem_size=D, elem_step=D,
                )
            if g == 0:
                nc.gpsimd.collective_compute(
                    "AllGather", mybir.AluOpType.bypass,
                    replica_groups=[list(range(M))],
                    ins=[z_local1.opt()], outs=[z_full1.opt()],
                )
            CgD = g2["Cg"][1][g]
            baseD = g2["ib"][1][g]
            gB = gy.tile([128, CGMAX, 128], bf16, name="g")
            for j in range(0, CgD, GCH):
                cg = min(GCH, CgD - j)
                b = baseD + j * 128
                nc.gpsimd.dma_gather(
                    gB[:, j : j + cg, :], z_full1[:],
                    idx_t["B2"][:, b // 16 : (b + cg * 128) // 16],
                    num_idxs=cg * 128, num_idxs_reg=cg * 128,
                    elem_size=D, elem_step=D,
                )
            for w in range(g * GW, min(WIN, (g + 1) * GW)):
                cs, ce = w * 128, (w + 1) * 128
                s_t = spool.tile([128, SW * 128], bf16, name="s")
                mms = build_s_and_cols(s_t, w, g, g2, erel2_t, gA, gB)
                p_d = ph.tile([128, 128], f32, name="p_h")
                nc.tensor.matmul(out=p_d[:], lhsT=hT0[:, cs:ce],
                                 rhs=wr2a_t[:], start=True, stop=False)
                nc.tensor.matmul(out=p_d[:], lhsT=hT1[:, cs:ce],
                                 rhs=wr2b_t[:], start=False, stop=True)
                p_o = pag.tile([128, 128], f32, name="p_agg")
                for i, (gt, tcol, scol) in enumerate(mms):
                    nc.tensor.matmul(
                        out=p_o[:],
                        lhsT=s_t[:, scol * 128 : (scol + 1) * 128],
                        rhs=gt[:, tcol, :],
                        start=(i == 0), stop=(i == len(mms) - 1),
                    )
                t1 = work.tile([128, 128], f32, name="t1")
                nc.vector.tensor_scalar(
                    t1[:], p_o[:], invc_t[:, w : w + 1], None, OP.mult)
                t2 = work.tile([128, 128], f32, name="t2")
                nc.vector.tensor_tensor(t2[:], t1[:], p_d[:], op=OP.add)
                ot = work.tile([128, 128], f32, name="ot")
                nc.vector.tensor_tensor(ot[:], t2[:], b2_t[:], op=OP.add)
                rows = min(128, NS - w * 128)
                nc.sync.dma_start(out_ext[w * 128 : w * 128 + rows, :],
                                  ot[:rows, :])

    nc.compile()
    return nc


def _host_prep(x, edge_index, W_l1, W_r1, b1, W_l2, W_r2, b2):
    bf = ml_dtypes.bfloat16
    x = np.ascontiguousarray(np.asarray(x, np.float32))
    ei = np.asarray(edge_index, np.int64)
    src, dst = ei[0], ei[1]

    cnt = np.bincount(dst, minlength=N).astype(np.float32)
    inv = (1.0 / np.maximum(cnt, 1.0)).astype(np.float32)

    c = dst // NS
    l = dst - c * NS
    w = l // 128
    rel = (l % 128).astype(np.int64)

    t1 = (src >= XSPLIT).astype(np.int64)
    i1 = src - XSPLIT * t1
    c2 = src // NS
    loc = src - c2 * NS
    t2 = (loc >= ZSPLIT).astype(np.int64)
    i2 = np.where(t2 == 0, c2 * ZSPLIT + loc, c2 * ZR1 + (loc - ZSPLIT))

    def layer_pack(t, tabidx):
        key = (c * WIN + w) * 2 + t
        order = np.argsort(key, kind="stable")
        n = np.bincount(key, minlength=M * WIN * 2)
        starts = np.concatenate([[0], np.cumsum(n)[:-1]])
        pos = np.arange(E) - starts[key[order]]
        n3 = n.reshape(M, WIN, 2)
        m = n3.max(axis=0)  # [WIN, 2] slot counts (max over cores)
        assert (m >= 128).all(), "window-table with <128 edges unsupported"
        ma, mb = [int(v) for v in m[:, 0]], [int(v) for v in m[:, 1]]
        gg = _layer_geom(ma, mb)
        oA = np.asarray(gg["o"][0])
        oB = np.asarray(gg["o"][1])
        ibA = np.asarray(gg["ib"][0])
        ibB = np.asarray(gg["ib"][1])
        cbA = np.asarray(gg["colbase"][0])
        cbB = np.asarray(gg["colbase"][1])
        LA = int(ibA[-1] + gg["Cg"][0][-1] * 128)
        LB = int(ibB[-1] + gg["Cg"][1][-1] * 128)

        so = order
        cs_, ws_, ts_ = c[so], w[so], t[so]
        g_ = ws_ // GW
        o = np.where(ts_ == 0, oA[ws_], oB[ws_])
        s_grp = o + pos                      # slot within group stream
        part = s_grp % 128
        col_l = s_grp // 128                 # column within group stream
        ib = np.where(ts_ == 0, ibA[g_], ibB[g_])
        gpos = ib + s_grp                    # slot within table idx stream
        cb = np.where(ts_ == 0, cbA[g_], cbB[g_])
        ecol = cb + col_l
        enc = rel[so] + 128 * (col_l * 128 < o)

        idxA = np.zeros((M, 16, LA // 16), np.int16)
        idxB = np.zeros((M, 16, LB // 16), np.int16)
        CT = gg["CT"]
        erel = np.full((M, 128, CT), -1.0, np.float32)
        ti_ = tabidx[so]
        for tt, arr in ((0, idxA), (1, idxB)):
            msk = ts_ == tt
            arr[cs_[msk], gpos[msk] % 16, gpos[msk] // 16] = ti_[msk].astype(
                np.int16)
        erel[cs_, part, ecol] = enc
        idxA = np.tile(idxA, (1, 8, 1))
        idxB = np.tile(idxB, (1, 8, 1))
        return ma, mb, idxA, idxB, erel

    ma1, mb1, idxA1, idxB1, erel1 = layer_pack(t1, i1)
    ma2, mb2, idxA2, idxB2, erel2 = layer_pack(t2, i2)
    g1 = _layer_geom(ma1, mb1)
    g2 = _layer_geom(ma2, mb2)
    JW = max(max(sa, sb) for gg in (g1, g2) for (sa, sb) in gg["spans"])

    xbf = x.astype(bf)
    xT = np.zeros((M, 128, NSP), np.float32)
    for cc in range(M):
        xT[cc, :, :NS] = x[cc * NS : (cc + 1) * NS].T

    invb = np.ones((M, NSP), np.float32)
    for cc in range(M):
        invb[cc, :NS] = inv[cc * NS : (cc + 1) * NS]
    invb_b = np.broadcast_to(invb[:, None, :], (M, 128, NSP))
    invc = invb.reshape(M, WIN, 128).transpose(0, 2, 1)  # [M,128,WIN]

    W_l1 = np.asarray(W_l1, np.float32).astype(bf)
    W_r1 = np.asarray(W_r1, np.float32).astype(bf)
    wl2 = np.asarray(W_l2, np.float32).astype(bf)
    wr2 = np.asarray(W_r2, np.float32).astype(bf)
    b1 = np.asarray(b1, np.float32)
    b1c = np.ascontiguousarray(np.stack([b1[:128], b1[128:]], axis=1))
    b2b = np.ascontiguousarray(
        np.tile(np.asarray(b2, np.float32)[None, :], (128, 1)))
    jcb = np.concatenate(
        [np.arange(128, 256, dtype=np.float32),
         np.tile(np.arange(128, dtype=np.float32), JW)]
    )
    jcb = np.ascontiguousarray(
        np.tile(jcb[None, :], (128, 1))).astype(bf)

    in_maps = []
    for cc in range(M):
        in_maps.append({
            "xbf": xbf,
            "xT": np.ascontiguousarray(xT[cc]).astype(bf),
            "idxA1": np.ascontiguousarray(idxA1[cc]),
            "idxB1": np.ascontiguousarray(idxB1[cc]),
            "idxA2": np.ascontiguousarray(idxA2[cc]),
            "idxB2": np.ascontiguousarray(idxB2[cc]),
            "erel1": np.ascontiguousarray(erel1[cc]).astype(bf),
            "erel2": np.ascontiguousarray(erel2[cc]).astype(bf),
            "jcb": jcb,
            "invb": np.ascontiguousarray(invb_b[cc]).astype(bf),
            "invc": np.ascontiguousarray(invc[cc]),
            "wl1": W_l1,
            "wr1": W_r1,
            "wl2": wl2,
            "wr2": wr2,
            "b1c": b1c,
            "b2b": b2b,
        })
    meta = (tuple(ma1), tuple(mb1), tuple(ma2), tuple(mb2))
    return in_maps, meta


def kernel(x, edge_index, W_l1, W_r1, b1, W_l2, W_r2, b2, _trace=False):
    from concourse import bass_utils

    in_maps, meta = _host_prep(x, edge_index, W_l1, W_r1, b1, W_l2, W_r2, b2)
    if meta not in _CACHE:
        _CACHE[meta] = _build(meta)
    nc = _CACHE[meta]
    res = bass_utils.run_bass_kernel_spmd(
        nc, in_maps, core_ids=list(range(M)), trace=_trace)
    out = np.concatenate([res.results[c]["out"] for c in range(M)], axis=0)
    if _trace:
        kernel.last_exec_time_ns = res.exec_time_ns
        kernel.last_results = res
    return out
